# revision 6
# baseline (speedup 1.0000x reference)
"""Block-sparse linear kernel for Trainium2 (8 NeuronCores, SPMD data-parallel).

Computes y = x @ (W * mask) + bias for
    x    [8, 1024, 4096] f32
    W    [4096, 4096]    f32
    mask [4096, 4096]    int32 (32x32-block structured, ~25% block density)
    bias [4096]          f32
    y    [8, 1024, 4096] f32

Strategy
--------
- Data parallel: core c computes rows [1024c, 1024(c+1)) of the flattened
  [8192, 4096] activation (i.e. batch element c).
- The trn2 PE array is physically 16 independent 32x32 sub-arrays; we run it
  in 64x32 tiling mode (8 concurrent sub-arrays).  The mask's 32x32 block
  granularity maps onto vertical block pairs: each present 64x32 "super
  cell" (block rows 2I,2I+1 x block col j, present if either 32x32 block is
  nonzero) becomes one K=64/M=32/N=512 matmul on sub-array
  (row_grp=I%2, col_grp=j%4); fully-zero super cells are skipped.
- v2: each supercell's weights are loaded into the PE array ONCE and used
  for both 512-token m-slices back to back.  The tile legalizer splits every
  matmul into LDWEIGHTS+MATMUL (marking the matmul non-self-loading); a
  post-schedule pass deletes the second, redundant LDWEIGHTS of each pair
  after verifying (against the final PE instruction order) that the weights
  for that sub-array position are already loaded.  This halves weight-load
  traffic on the PE weight path and lets weights stream from HBM once
  (previously twice: once per m-slice pass).
- Ramp: the first GEN_J supertiles run in rounds of 2, their matmuls merged
  chunk-major (all supertiles' blocks for x chunk t before chunk t+1, both
  m-slices interleaved), so early compute tracks x-chunk DMA arrival instead
  of head-of-line blocking on a late chunk.  x chunks are DMA'd m0/m1
  interleaved per chunk to match.
- Weights are gathered host-side into per-row-strip BSR-style panels (this
  mirrors the nn.Module, which stores BSR values at init), cast to bf16;
  x is transposed/cast host-side.  All matmul FLOPs run in bf16 with fp32
  PSUM accumulation (measured rel. error ~2e-3).
- The device program is compiled against the observed block pattern; it is
  exact for arbitrary masks (any block containing a nonzero mask element is
  gathered with its W*mask values; absent blocks contribute exact zeros).
"""

import numpy as np
import ml_dtypes

B, S, IN_F, OUT_F = 8, 1024, 4096, 4096
BS = 32                      # sparsity block size
GI, GJ = IN_F // BS, OUT_F // BS
GP = GI // 2                 # vertical super-rows (64 rows each)
N_CORES = 8
M_CORE = (B * S) // N_CORES  # rows of x per core (1024)
MSL = 512                    # m-slice width (one PSUM bank of fp32)
N_MSL = M_CORE // MSL        # 2
JCOLS = 4                    # output block-columns per supertile (4*32 = 128 partitions)
N_J = GJ // JCOLS            # 32 output supertiles
N_T = IN_F // 128            # 32 xT tiles
GEN_J = 8                    # supertiles run chunk-major in rounds of 2

BF16 = ml_dtypes.bfloat16


def _ensure_ntff_hook():
    """Best-effort: make trace=True work under axon when the image's antenv
    lacks axon_hooks.  Harmless if it fails — tracing is skipped, results
    are still correct."""
    import sys, types
    try:
        import antenv  # noqa
    except ImportError:
        return
    try:
        from antenv.axon_hooks import get_axon_ntff_profile_hook
        if get_axon_ntff_profile_hook() is not None:
            return
        mod = sys.modules["antenv.axon_hooks"]
    except ImportError:
        mod = types.ModuleType("antenv.axon_hooks")
        mod._hook = None
        def set_axon_ntff_profile_hook(h, _m=mod):
            _m._hook = h
        def get_axon_ntff_profile_hook(_m=mod):
            return _m._hook
        mod.set_axon_ntff_profile_hook = set_axon_ntff_profile_hook
        mod.get_axon_ntff_profile_hook = get_axon_ntff_profile_hook
        sys.modules["antenv.axon_hooks"] = mod
        import antenv as _a
        _a.axon_hooks = mod
    try:
        from trn_agent_boot.trn_boot import _ntff_profile_via_ctypes
        mod.set_axon_ntff_profile_hook(
            _ntff_profile_via_ctypes("/opt/axon/libaxon_pjrt.so")
        )
    except Exception:
        pass


def _pair_permutation(nzb):
    """Order block-rows so vertically-paired rows co-occur in many columns.

    Greedy max-weight matching on C[a,b] = #columns where blocks a and b are
    both present; each matched pair becomes one 64-row super-row, so high
    weight = fewer half-empty 64x32 panels = fewer matmuls.
    """
    C = nzb.astype(np.int32) @ nzb.astype(np.int32).T
    pairs = []
    try:
        import networkx as nx
        G = nx.Graph()
        for a in range(GI):
            for b in range(a + 1, GI):
                G.add_edge(a, b, weight=int(C[a, b]))
        pairs = [
            (int(min(a, b)), int(max(a, b)))
            for a, b in nx.max_weight_matching(G, maxcardinality=True)
        ]
    except Exception:
        pass
    if len(pairs) != GI // 2:
        pairs = []
        iu = np.triu_indices(GI, k=1)
        order = np.argsort(C[iu])[::-1]
        used = np.zeros(GI, dtype=bool)
        for idx in order:
            a, b = iu[0][idx], iu[1][idx]
            if not used[a] and not used[b]:
                used[a] = used[b] = True
                pairs.append((int(a), int(b)))
                if len(pairs) == GI // 2:
                    break
    perm = []
    for a, b in pairs:
        perm.extend((a, b))
    for a in range(GI):      # safety for odd leftovers
        if a not in perm:
            perm.append(a)
    return np.asarray(perm)


def _plan(nzb):
    """Per-supertile weight storage layout and MM schedule (64x32 pairing).

    nzb: bool [GI, GJ] — which 32x32 blocks are present (in permuted row
    order).

    Returns (plan, strip_cols):
      plan[J] = {
        'chunks': {r2: (src_col_base, n_cells)},            # DMA per row strip
        'sched':  [(r2, c, woff_or_None, I, start, stop)],
      }
      strip_cols[r2] = total columns of strip r2's DRAM panel (r2 in {0,1}).
    woff None => dummy matmul with the zero-weight tile (region had no cells
    but must be initialized so the bank reduce reads defined values).
    """
    nzb2 = nzb[0::2] | nzb[1::2]       # [GP, GJ] supercell presence
    plan = []
    strip_cols = [0, 0]
    for J in range(N_J):
        per_strip = {0: [], 1: []}     # storage order: x-tile-ascending so the
        for I in range(GP):            # ramp consumes x chunks as they arrive
            for j in range(J * JCOLS, (J + 1) * JCOLS):
                if nzb2[I, j]:
                    per_strip[I % 2].append((I, j))
        chunks = {}
        queues = {}                    # (r2, c) -> list of (r2, c, woff, I)
        for r2 in range(2):
            cells = per_strip[r2]
            chunks[r2] = (strip_cols[r2], len(cells))
            strip_cols[r2] += len(cells) * BS
            for k, (I, j) in enumerate(cells):
                c = j % 4
                queues.setdefault((r2, c), []).append((r2, c, k * BS, I))
        for r2 in range(2):
            for c in range(4):
                if (r2, c) not in queues:
                    queues[(r2, c)] = [(r2, c, None, 0)]
        # Round-robin across the 8 sub-array positions for concurrency.
        sched = []
        qlists = [queues[k] for k in sorted(queues.keys())]
        idx = [0] * len(qlists)
        remaining = sum(len(q) for q in qlists)
        while remaining:
            for qi, q in enumerate(qlists):
                if idx[qi] < len(q):
                    r2, c, woff, I = q[idx[qi]]
                    start = idx[qi] == 0
                    stop = idx[qi] == len(q) - 1
                    sched.append((r2, c, woff, I, start, stop))
                    idx[qi] += 1
                    remaining -= 1
        plan.append({"chunks": chunks, "sched": sched})
    return plan, strip_cols


def _elide_redundant_ldweights(nc, candidates):
    """Delete LDWEIGHTS whose weights are provably already loaded.

    Walks each basic block's final (scheduled) PE instruction stream,
    tracking per tile_position the access-pattern of the last kept
    LDWEIGHTS.  An LDWEIGHTS is deleted iff (a) the matmul it precedes is a
    marked candidate (the m1 twin of an identically-weighted m0 matmul) and
    (b) the tracked state for its position already equals its weights AP.
    Waits/updates on a deleted LDWEIGHTS move onto its matmul; dep-graph
    descendant references are repointed.  This is order-verified: if the
    scheduler separated a pair, the state check fails and the load is kept.
    """
    import concourse.mybir as mybir

    n_removed = 0
    n_kept_cand = 0
    renames = {}
    for bb in nc.main_func.blocks:
        insts = list(bb.instructions)
        pe = [
            (i, x)
            for i, x in enumerate(insts)
            if x.engine == mybir.EngineType.PE
        ]
        state = {}
        dead = []
        for k, (idx, inst) in enumerate(pe):
            if not isinstance(inst, mybir.InstLdweights):
                continue
            pos = inst.tile_position
            aps = str(inst.ins[0])
            mm = pe[k + 1][1] if k + 1 < len(pe) else None
            if (
                mm is not None
                and type(mm).__name__ == "InstMatmult"
                and mm.name in candidates
            ):
                if state.get(pos) == aps:
                    si = inst.sync_info
                    if si is not None and (si.on_wait or si.on_update):
                        msi = mm.sync_info
                        if msi is None:
                            mm.sync_info = mybir.SyncInfo(
                                on_wait=list(si.on_wait),
                                on_update=list(si.on_update),
                            )
                        else:
                            mm.sync_info = mybir.SyncInfo(
                                on_wait=list(si.on_wait) + list(msi.on_wait),
                                on_update=list(msi.on_update)
                                + list(si.on_update),
                            )
                    dead.append((idx, inst))
                    renames[inst.name] = mm.name
                    continue
                n_kept_cand += 1
            state[pos] = aps
        for idx, inst in sorted(dead, key=lambda t: -t[0]):
            del bb.instructions[idx]
            nc.inst_map.pop(inst.name, None)
            n_removed += 1
    if renames:
        dead_names = set(renames)
        for name, inst in nc.inst_map.items():
            d = inst.descendants
            if d:
                hit = dead_names.intersection(d)
                for old in hit:
                    d.discard(old)
                    d.add(renames[old])
    return n_removed, n_kept_cand


def _build_program(plan, strip_cols):
    import concourse.bacc as bacc
    import concourse.tile as tile
    import concourse.mybir as mybir

    nc = bacc.Bacc(debug=False)
    bf16, f32 = mybir.dt.bfloat16, mybir.dt.float32

    xt_d = nc.declare_dram_parameter(
        "xt", [N_MSL * N_T, 128, MSL], bf16, isOutput=False
    )
    w_d = {}
    for r2 in range(2):
        if strip_cols[r2] > 0:
            w_d[r2] = nc.declare_dram_parameter(
                f"w{r2}", [2 * BS, strip_cols[r2]], bf16, isOutput=False
            )
    out_d = nc.declare_dram_parameter("out", [OUT_F, M_CORE], f32, isOutput=True)

    # Largest per-(J, strip) weight chunk, in columns (>= BS for the tile alloc).
    lmax = BS
    for p in plan:
        for r2 in range(2):
            lmax = max(lmax, p["chunks"][r2][1] * BS)

    elide = set()

    with tile.TileContext(nc) as tc:
        with (
            tc.tile_pool(name="xp", bufs=1) as xp,
            tc.tile_pool(name="zp", bufs=1) as zp,
            tc.tile_pool(name="wp", bufs=10) as wp,
            tc.tile_pool(name="ep", bufs=10) as ep,
            tc.tile_pool(name="pp", bufs=4, space="PSUM") as pp,
        ):
            def load_w(J):
                wt = wp.tile([128, lmax], bf16, tag="wt")
                for r2 in range(2):
                    base, ncell = plan[J]["chunks"][r2]
                    if ncell:
                        nc.scalar.dma_start(
                            wt[64 * r2 : 64 * r2 + 64, : ncell * BS],
                            w_d[r2][:, base : base + ncell * BS],
                        )
                return wt

            Xc = {}

            def load_x_chunk(t, m, eng):
                xchunk = xp.tile([128, MSL], bf16, tag=f"x{t}_{m}")
                Xc[(t, m)] = xchunk
                eng.dma_start(xchunk[:], xt_d[m * N_T + t])

            # DMA emission order (per in-order queue): weights on the scalar
            # queue, J-ascending; x chunks m0/m1-interleaved per chunk, even
            # chunks on sync, odd on gpsimd.  First two supertiles' weights
            # lead so the very first matmuls' inputs all land ASAP.
            zw = zp.tile([128, BS], bf16)
            nc.vector.memset(zw[:], 0.0)
            wts = {}
            wts[0] = load_w(0)
            wts[1] = load_w(1)
            for t in range(2):
                for m in range(N_MSL):
                    load_x_chunk(t, m, (nc.sync, nc.gpsimd)[t % 2])
            for J in range(2, N_J):
                wts[J] = load_w(J)
                for t in (2 * J - 2, 2 * J - 1):
                    if t < N_T:
                        for m in range(N_MSL):
                            load_x_chunk(t, m, (nc.sync, nc.gpsimd)[t % 2])

            def emit_mm(P, wt, r2, c, woff, I, m, start, stop):
                lhsT = (
                    zw[64 * r2 : 64 * r2 + 64, :]
                    if woff is None
                    else wt[64 * r2 : 64 * r2 + 64, woff : woff + BS]
                )
                return nc.tensor.matmul(
                    P[32 * c : 32 * c + 32, r2, :],
                    lhsT,
                    Xc[(I // 2, m)][64 * r2 : 64 * r2 + 64, :],
                    start=start,
                    stop=stop,
                    tile_position=(64 * r2, 32 * c),
                )

            n_evac = [0]

            def emit_evac(P, J, m):
                ob = ep.tile([128, MSL], f32, tag="ob")
                nc.vector.reduce_sum(
                    ob[:], P[:].transpose([0, 2, 1]), axis=mybir.AxisListType.X
                )
                (nc.sync, nc.gpsimd)[n_evac[0] % 2].dma_start(
                    out_d[128 * J : 128 * (J + 1), m * MSL : (m + 1) * MSL],
                    ob[:],
                )
                n_evac[0] += 1

            # GEN rounds: 2 supertiles at a time, merged chunk-major (all
            # blocks needing x chunk t — both m-slices — before chunk t+1),
            # so compute tracks x-chunk DMA arrival.  Uses all 8 PSUM banks
            # (2 J x 2 m-slices x 2 banks).
            for r0 in range(0, GEN_J, 2):
                ROUND = [r0, r0 + 1]
                merged = []
                for J in ROUND:
                    for k, (r2, c, woff, I, _s0, _s1) in enumerate(
                        plan[J]["sched"]
                    ):
                        t = -1 if woff is None else I // 2
                        merged.append((t, k, J, r2, c, woff, I))
                merged.sort(key=lambda e: (e[0], e[1], e[2]))
                first_of = {}
                last_of = {}
                for idx, e in enumerate(merged):
                    key = (e[2], e[3], e[4])
                    first_of.setdefault(key, idx)
                    last_of[key] = idx
                P0 = {
                    J: pp.tile([128, 2, MSL], f32, tag="P", name=f"Pg0_{J}")
                    for J in ROUND
                }
                P1 = {
                    J: pp.tile([128, 2, MSL], f32, tag="P", name=f"Pg1_{J}")
                    for J in ROUND
                }
                for idx, (t, k, J, r2, c, woff, I) in enumerate(merged):
                    key = (J, r2, c)
                    st = first_of[key] == idx
                    sp = last_of[key] == idx
                    emit_mm(P0[J], wts[J], r2, c, woff, I, 0, st, sp)
                    mm1 = emit_mm(P1[J], wts[J], r2, c, woff, I, 1, st, sp)
                    elide.add(mm1.ins.name)
                for J in ROUND:
                    emit_evac(P0[J], J, 0)
                    emit_evac(P1[J], J, 1)

            # Steady phase: per supertile, each supercell's weights loaded
            # once, both m-slice matmuls back to back (second load elided).
            for J in range(GEN_J, N_J):
                P0 = pp.tile([128, 2, MSL], f32, tag="P")
                P1 = pp.tile([128, 2, MSL], f32, tag="P")
                for r2, c, woff, I, start, stop in plan[J]["sched"]:
                    emit_mm(P0, wts[J], r2, c, woff, I, 0, start, stop)
                    mm1 = emit_mm(P1, wts[J], r2, c, woff, I, 1, start, stop)
                    elide.add(mm1.ins.name)
                emit_evac(P0, J, 0)
                emit_evac(P1, J, 1)

    n_removed, n_kept = _elide_redundant_ldweights(nc, elide)
    _build_program.elide_stats = (n_removed, n_kept, len(elide))
    print(
        f"[kernel] ldweights elided {n_removed}, kept-candidates {n_kept}, "
        f"candidates {len(elide)}"
    )
    nc.compile()
    return nc


_CACHE = {}


def kernel(x, W, bias, mask):
    assert x.shape == (B, S, IN_F) and W.shape == (IN_F, OUT_F)
    _ensure_ntff_hook()
    from concourse.bass_utils import run_bass_kernel_spmd

    # --- host-side input prep -------------------------------------------
    mask_nz = mask != 0
    nzb = np.asarray(mask_nz.reshape(GI, BS, GJ, BS).any(axis=(1, 3)))

    key = nzb.tobytes()
    if key not in _CACHE:
        perm = _pair_permutation(nzb)
        plan, strip_cols = _plan(nzb[perm])
        nc = _build_program(plan, strip_cols)
        _CACHE[key] = (perm, plan, strip_cols, nc)
    perm, plan, strip_cols, nc = _CACHE[key]
    nzb_p = nzb[perm]

    # Masked weights, gathered per row strip in storage order (J-major).
    # Wm's zeros for absent 32x32 blocks make half-present 64x32 panels
    # correct with no special-casing.
    Wm = np.where(mask_nz, W, np.float32(0)).astype(np.float32)
    W4 = Wm.reshape(GI, BS, GJ, BS)  # block (i, j) = W4[i, :, j, :]
    nzb2 = nzb_p[0::2] | nzb_p[1::2]
    strips = {}
    for r2 in range(2):
        if strip_cols[r2] == 0:
            continue
        II, JJ = [], []
        for J in range(N_J):
            for I in range(GP):
                for j in range(J * JCOLS, (J + 1) * JCOLS):
                    if nzb2[I, j] and I % 2 == r2:
                        II.append(I)
                        JJ.append(j)
        II = np.asarray(II)
        JJ = np.asarray(JJ)
        top = W4[perm[2 * II], :, JJ, :]       # [n, 32, 32]
        bot = W4[perm[2 * II + 1], :, JJ, :]   # [n, 32, 32]
        panel = np.concatenate([top, bot], axis=1)  # [n, 64, 32]
        strips[r2] = np.ascontiguousarray(
            panel.transpose(1, 0, 2).reshape(2 * BS, -1)
        ).astype(BF16)

    xf = np.ascontiguousarray(x).reshape(B * S, IN_F)
    in_maps = []
    for c in range(N_CORES):
        xt = np.ascontiguousarray(
            xf[c * M_CORE : (c + 1) * M_CORE].T
        ).astype(BF16)
        xt = xt.reshape(GI, BS, M_CORE)[perm].reshape(IN_F, M_CORE)
        xtc = (
            xt.reshape(N_T, 128, N_MSL, MSL)
            .transpose(2, 0, 1, 3)
            .reshape(N_MSL * N_T, 128, MSL)
        )
        m = {"xt": np.ascontiguousarray(xtc)}
        for r2, arr in strips.items():
            m[f"w{r2}"] = arr
        in_maps.append(m)

    # --- run -------------------------------------------------------------
    res = run_bass_kernel_spmd(nc, in_maps, list(range(N_CORES)), trace=True)

    # --- host-side output assembly --------------------------------------
    y = np.empty((B * S, OUT_F), dtype=np.float32)
    for c in range(N_CORES):
        y[c * M_CORE : (c + 1) * M_CORE] = res.results[c]["out"].T
    y = y.reshape(B, S, OUT_F)
    if np.any(bias):
        # bias is all-zero in this problem's setup; handled host-side for
        # generality.
        y = y + bias.astype(np.float32)
    kernel.last_exec_time_ns = res.exec_time_ns
    return y


# revision 10
# speedup vs baseline: 1.2444x; 1.2444x over previous
"""Block-sparse linear kernel for Trainium2 (8 NeuronCores, SPMD data-parallel).

Computes y = x @ (W * mask) + bias for
    x    [8, 1024, 4096] f32
    W    [4096, 4096]    f32
    mask [4096, 4096]    int32 (32x32-block structured, ~25% block density)
    bias [4096]          f32
    y    [8, 1024, 4096] f32

Strategy
--------
- Data parallel: core c computes rows [1024c, 1024(c+1)) of the flattened
  [8192, 4096] activation (i.e. batch element c).
- The trn2 PE array is physically 16 independent 32x32 sub-arrays; we run it
  in 64x32 tiling mode (8 concurrent sub-arrays).  The mask's 32x32 block
  granularity maps onto vertical block pairs: each present 64x32 "super
  cell" (block rows 2I,2I+1 x block col j, present if either 32x32 block is
  nonzero) becomes one K=64/M=32/N=512 matmul on sub-array
  (row_grp=I%2, col_grp=j%4); fully-zero super cells are skipped.
- v2: each supercell's weights are loaded into the PE array ONCE and used
  for both 512-token m-slices back to back.  The tile legalizer splits every
  matmul into LDWEIGHTS+MATMUL (marking the matmul non-self-loading); a
  post-schedule pass deletes the second, redundant LDWEIGHTS of each pair
  after verifying (against the final PE instruction order) that the weights
  for that sub-array position are already loaded.  This halves weight-load
  traffic on the PE weight path and lets weights stream from HBM once
  (previously twice: once per m-slice pass).
- Ramp: the first GEN_J supertiles run in rounds of 2, their matmuls merged
  chunk-major (all supertiles' blocks for x chunk t before chunk t+1, both
  m-slices interleaved), so early compute tracks x-chunk DMA arrival instead
  of head-of-line blocking on a late chunk.  x chunks are DMA'd m0/m1
  interleaved per chunk to match.
- Weights are gathered host-side into per-row-strip BSR-style panels (this
  mirrors the nn.Module, which stores BSR values at init), cast to bf16;
  x is transposed/cast host-side.  All matmul FLOPs run in bf16 with fp32
  PSUM accumulation (measured rel. error ~2e-3).
- The device program is compiled against the observed block pattern; it is
  exact for arbitrary masks (any block containing a nonzero mask element is
  gathered with its W*mask values; absent blocks contribute exact zeros).
"""

import numpy as np
import ml_dtypes

B, S, IN_F, OUT_F = 8, 1024, 4096, 4096
BS = 32                      # sparsity block size
GI, GJ = IN_F // BS, OUT_F // BS
GP = GI // 2                 # vertical super-rows (64 rows each)
N_CORES = 8
M_CORE = (B * S) // N_CORES  # rows of x per core (1024)
MSL = 512                    # m-slice width (one PSUM bank of fp32)
N_MSL = M_CORE // MSL        # 2
JCOLS = 4                    # output block-columns per supertile (4*32 = 128 partitions)
N_J = GJ // JCOLS            # 32 output supertiles
N_T = IN_F // 128            # 32 xT tiles
GEN_J = 8                    # supertiles run chunk-major in rounds of 2

BF16 = ml_dtypes.bfloat16


def _ensure_ntff_hook():
    """Best-effort: make trace=True work under axon when the image's antenv
    lacks axon_hooks.  Harmless if it fails — tracing is skipped, results
    are still correct."""
    import sys, types
    try:
        import antenv  # noqa
    except ImportError:
        return
    try:
        from antenv.axon_hooks import get_axon_ntff_profile_hook
        if get_axon_ntff_profile_hook() is not None:
            return
        mod = sys.modules["antenv.axon_hooks"]
    except ImportError:
        mod = types.ModuleType("antenv.axon_hooks")
        mod._hook = None
        def set_axon_ntff_profile_hook(h, _m=mod):
            _m._hook = h
        def get_axon_ntff_profile_hook(_m=mod):
            return _m._hook
        mod.set_axon_ntff_profile_hook = set_axon_ntff_profile_hook
        mod.get_axon_ntff_profile_hook = get_axon_ntff_profile_hook
        sys.modules["antenv.axon_hooks"] = mod
        import antenv as _a
        _a.axon_hooks = mod
    try:
        from trn_agent_boot.trn_boot import _ntff_profile_via_ctypes
        mod.set_axon_ntff_profile_hook(
            _ntff_profile_via_ctypes("/opt/axon/libaxon_pjrt.so")
        )
    except Exception:
        pass


def _pair_permutation(nzb):
    """Order block-rows so vertically-paired rows co-occur in many columns.

    Greedy max-weight matching on C[a,b] = #columns where blocks a and b are
    both present; each matched pair becomes one 64-row super-row, so high
    weight = fewer half-empty 64x32 panels = fewer matmuls.
    """
    C = nzb.astype(np.int32) @ nzb.astype(np.int32).T
    pairs = []
    try:
        import networkx as nx
        G = nx.Graph()
        for a in range(GI):
            for b in range(a + 1, GI):
                G.add_edge(a, b, weight=int(C[a, b]))
        pairs = [
            (int(min(a, b)), int(max(a, b)))
            for a, b in nx.max_weight_matching(G, maxcardinality=True)
        ]
    except Exception:
        pass
    if len(pairs) != GI // 2:
        pairs = []
        iu = np.triu_indices(GI, k=1)
        order = np.argsort(C[iu])[::-1]
        used = np.zeros(GI, dtype=bool)
        for idx in order:
            a, b = iu[0][idx], iu[1][idx]
            if not used[a] and not used[b]:
                used[a] = used[b] = True
                pairs.append((int(a), int(b)))
                if len(pairs) == GI // 2:
                    break
    perm = []
    for a, b in pairs:
        perm.extend((a, b))
    for a in range(GI):      # safety for odd leftovers
        if a not in perm:
            perm.append(a)
    return np.asarray(perm)


def _plan(nzb):
    """Per-supertile weight storage layout and MM schedule (64x32 pairing).

    nzb: bool [GI, GJ] — which 32x32 blocks are present (in permuted row
    order).

    Returns (plan, strip_cols):
      plan[J] = {
        'chunks': {r2: (src_col_base, n_cells)},            # DMA per row strip
        'sched':  [(r2, c, woff_or_None, I, start, stop)],
      }
      strip_cols[r2] = total columns of strip r2's DRAM panel (r2 in {0,1}).
    woff None => dummy matmul with the zero-weight tile (region had no cells
    but must be initialized so the bank reduce reads defined values).
    """
    nzb2 = nzb[0::2] | nzb[1::2]       # [GP, GJ] supercell presence
    plan = []
    strip_cols = [0, 0]
    for J in range(N_J):
        per_strip = {0: [], 1: []}     # storage order: x-tile-ascending so the
        for I in range(GP):            # ramp consumes x chunks as they arrive
            for j in range(J * JCOLS, (J + 1) * JCOLS):
                if nzb2[I, j]:
                    per_strip[I % 2].append((I, j))
        chunks = {}
        queues = {}                    # (r2, c) -> list of (r2, c, woff, I)
        for r2 in range(2):
            cells = per_strip[r2]
            chunks[r2] = (strip_cols[r2], len(cells))
            strip_cols[r2] += len(cells) * BS
            for k, (I, j) in enumerate(cells):
                c = j % 4
                queues.setdefault((r2, c), []).append((r2, c, k * BS, I))
        for r2 in range(2):
            for c in range(4):
                if (r2, c) not in queues:
                    queues[(r2, c)] = [(r2, c, None, 0)]
        # Round-robin across the 8 sub-array positions for concurrency.
        sched = []
        qlists = [queues[k] for k in sorted(queues.keys())]
        idx = [0] * len(qlists)
        remaining = sum(len(q) for q in qlists)
        while remaining:
            for qi, q in enumerate(qlists):
                if idx[qi] < len(q):
                    r2, c, woff, I = q[idx[qi]]
                    start = idx[qi] == 0
                    stop = idx[qi] == len(q) - 1
                    sched.append((r2, c, woff, I, start, stop))
                    idx[qi] += 1
                    remaining -= 1
        plan.append({"chunks": chunks, "sched": sched})
    return plan, strip_cols


def _elide_redundant_ldweights(nc, candidates):
    """Delete LDWEIGHTS whose weights are provably already loaded.

    Walks each basic block's final (scheduled) PE instruction stream,
    tracking per tile_position the access-pattern of the last kept
    LDWEIGHTS.  An LDWEIGHTS is deleted iff (a) the matmul it precedes is a
    marked candidate (the m1 twin of an identically-weighted m0 matmul) and
    (b) the tracked state for its position already equals its weights AP.
    Waits/updates on a deleted LDWEIGHTS move onto its matmul; dep-graph
    descendant references are repointed.  This is order-verified: if the
    scheduler separated a pair, the state check fails and the load is kept.
    """
    import concourse.mybir as mybir

    n_removed = 0
    n_kept_cand = 0
    renames = {}
    for bb in nc.main_func.blocks:
        insts = list(bb.instructions)
        pe = [
            (i, x)
            for i, x in enumerate(insts)
            if x.engine == mybir.EngineType.PE
        ]
        state = {}
        dead = []
        for k, (idx, inst) in enumerate(pe):
            if not isinstance(inst, mybir.InstLdweights):
                continue
            pos = inst.tile_position
            aps = str(inst.ins[0])
            mm = pe[k + 1][1] if k + 1 < len(pe) else None
            if (
                mm is not None
                and type(mm).__name__ == "InstMatmult"
                and mm.name in candidates
            ):
                if state.get(pos) == aps:
                    si = inst.sync_info
                    if si is not None and (si.on_wait or si.on_update):
                        msi = mm.sync_info
                        if msi is None:
                            mm.sync_info = mybir.SyncInfo(
                                on_wait=list(si.on_wait),
                                on_update=list(si.on_update),
                            )
                        else:
                            mm.sync_info = mybir.SyncInfo(
                                on_wait=list(si.on_wait) + list(msi.on_wait),
                                on_update=list(msi.on_update)
                                + list(si.on_update),
                            )
                    dead.append((idx, inst))
                    renames[inst.name] = mm.name
                    continue
                n_kept_cand += 1
            state[pos] = aps
        for idx, inst in sorted(dead, key=lambda t: -t[0]):
            del bb.instructions[idx]
            nc.inst_map.pop(inst.name, None)
            n_removed += 1
    if renames:
        dead_names = set(renames)
        for name, inst in nc.inst_map.items():
            d = inst.descendants
            if d:
                hit = dead_names.intersection(d)
                for old in hit:
                    d.discard(old)
                    d.add(renames[old])
    return n_removed, n_kept_cand


def _build_program(plan, strip_cols):
    import concourse.bacc as bacc
    import concourse.tile as tile
    import concourse.mybir as mybir

    nc = bacc.Bacc(debug=False)
    bf16, f32 = mybir.dt.bfloat16, mybir.dt.float32

    xt_d = nc.declare_dram_parameter(
        "xt", [N_MSL * N_T, 128, MSL], bf16, isOutput=False
    )
    w_d = {}
    for r2 in range(2):
        if strip_cols[r2] > 0:
            w_d[r2] = nc.declare_dram_parameter(
                f"w{r2}", [2 * BS, strip_cols[r2]], bf16, isOutput=False
            )
    out_d = nc.declare_dram_parameter("out", [OUT_F, M_CORE], f32, isOutput=True)

    # Largest per-(J, strip) weight chunk, in columns (>= BS for the tile alloc).
    lmax = BS
    for p in plan:
        for r2 in range(2):
            lmax = max(lmax, p["chunks"][r2][1] * BS)

    elide = set()

    with tile.TileContext(nc) as tc:
        with (
            tc.tile_pool(name="xp", bufs=1) as xp,
            tc.tile_pool(name="zp", bufs=1) as zp,
            tc.tile_pool(name="wp", bufs=10) as wp,
            tc.tile_pool(name="ep", bufs=10) as ep,
            tc.tile_pool(name="pp", bufs=4, space="PSUM") as pp,
        ):
            def load_w(J, engs=None):
                wt = wp.tile([128, lmax], bf16, tag="wt")
                for r2 in range(2):
                    base, ncell = plan[J]["chunks"][r2]
                    if ncell:
                        eng = nc.scalar if engs is None else engs[r2 % len(engs)]
                        eng.dma_start(
                            wt[64 * r2 : 64 * r2 + 64, : ncell * BS],
                            w_d[r2][:, base : base + ncell * BS],
                        )
                return wt

            Xc = {}

            def load_x_chunk(t, m, eng):
                xchunk = xp.tile([128, MSL], bf16, tag=f"x{t}_{m}")
                Xc[(t, m)] = xchunk
                eng.dma_start(xchunk[:], xt_d[m * N_T + t])

            # DMA emission order: the GEN supertiles' weights lead, then ALL
            # of x m-slice 0 round-robin across the three input queues (the
            # m0 generation sweep tracks its arrival), then x m-slice 1, then
            # the remaining weights.  Per-queue order follows emission.
            QS = (nc.sync, nc.gpsimd, nc.scalar)
            zw = zp.tile([128, BS], bf16)
            nc.vector.memset(zw[:], 0.0)
            wts = {}
            for J in range(4):
                wts[J] = load_w(J)
            for m in range(N_MSL):
                for t in range(N_T):
                    load_x_chunk(t, m, QS[t % 3])
            for J in range(4, N_J):
                wts[J] = load_w(J, engs=(QS[J % 3], QS[(J + 1) % 3]))

            def emit_mm(P, wt, r2, c, woff, I, m, start, stop):
                lhsT = (
                    zw[64 * r2 : 64 * r2 + 64, :]
                    if woff is None
                    else wt[64 * r2 : 64 * r2 + 64, woff : woff + BS]
                )
                return nc.tensor.matmul(
                    P[32 * c : 32 * c + 32, r2, :],
                    lhsT,
                    Xc[(I // 2, m)][64 * r2 : 64 * r2 + 64, :],
                    start=start,
                    stop=stop,
                    tile_position=(64 * r2, 32 * c),
                )

            n_evac = [0]

            def emit_evac(P, J, m):
                ob = ep.tile([128, MSL], f32, tag="ob")
                nc.vector.reduce_sum(
                    ob[:], P[:].transpose([0, 2, 1]), axis=mybir.AxisListType.X
                )
                (nc.sync, nc.gpsimd)[n_evac[0] % 2].dma_start(
                    out_d[128 * J : 128 * (J + 1), m * MSL : (m + 1) * MSL],
                    ob[:],
                )
                n_evac[0] += 1

            # GEN: the first 4 supertiles' m-slice sweeps run chunk-major
            # merged (all four tiles' blocks for x chunk t before any of
            # chunk t+1), so early compute tracks x-chunk DMA arrival.  The
            # m0 sweep runs first (x m0 is DMA'd first), then the m1 sweep
            # (self-loading: array weights were clobbered in between).
            GEN = list(range(4))
            merged = []
            for J in GEN:
                for k, (r2, c, woff, I, _s0, _s1) in enumerate(plan[J]["sched"]):
                    t = -1 if woff is None else I // 2
                    merged.append((t, k, J, r2, c, woff, I))
            merged.sort(key=lambda e: (e[0], e[1], e[2]))
            first_of = {}
            last_of = {}
            for idx, e in enumerate(merged):
                key = (e[2], e[3], e[4])
                first_of.setdefault(key, idx)
                last_of[key] = idx
            for m in range(N_MSL):
                Pg = {
                    J: pp.tile([128, 2, MSL], f32, tag="P", name=f"Pg{m}_{J}")
                    for J in GEN
                }
                for idx, (t, k, J, r2, c, woff, I) in enumerate(merged):
                    key = (J, r2, c)
                    emit_mm(
                        Pg[J], wts[J], r2, c, woff, I, m,
                        first_of[key] == idx, last_of[key] == idx,
                    )
                for J in GEN:
                    emit_evac(Pg[J], J, m)

            # Steady phase: per supertile, each supercell's weights are
            # loaded into the PE array once; the m1 matmul is emitted LAG
            # entries after its m0 twin so it lands on a different sub-array
            # position (positions rotate with period 8) — back-to-back
            # same-position matmuls would serialize on the sub-array, since
            # matmul starts are pc-monotone.  LAG < 8 keeps the weight state
            # intact for the elision pass.
            LAG = 6
            pend = []

            def pop_m1():
                P1, wt, (r2, c, woff, I, st, sp) = pend.pop(0)
                mm1 = emit_mm(P1, wt, r2, c, woff, I, 1, st, sp)
                elide.add(mm1.ins.name)

            for J in range(len(GEN), N_J):
                P0 = pp.tile([128, 2, MSL], f32, tag="P", name=f"P0_{J}")
                P1 = pp.tile([128, 2, MSL], f32, tag="P", name=f"P1_{J}")
                for entry in plan[J]["sched"]:
                    r2, c, woff, I, start, stop = entry
                    emit_mm(P0, wts[J], r2, c, woff, I, 0, start, stop)
                    pend.append((P1, wts[J], entry))
                    if len(pend) > LAG:
                        pop_m1()
                while pend:
                    pop_m1()
                emit_evac(P0, J, 0)
                emit_evac(P1, J, 1)

    n_removed, n_kept = _elide_redundant_ldweights(nc, elide)
    _build_program.elide_stats = (n_removed, n_kept, len(elide))
    print(
        f"[kernel] ldweights elided {n_removed}, kept-candidates {n_kept}, "
        f"candidates {len(elide)}"
    )
    nc.compile()
    return nc


_CACHE = {}


def kernel(x, W, bias, mask):
    assert x.shape == (B, S, IN_F) and W.shape == (IN_F, OUT_F)
    _ensure_ntff_hook()
    from concourse.bass_utils import run_bass_kernel_spmd

    # --- host-side input prep -------------------------------------------
    mask_nz = mask != 0
    nzb = np.asarray(mask_nz.reshape(GI, BS, GJ, BS).any(axis=(1, 3)))

    key = nzb.tobytes()
    if key not in _CACHE:
        perm = _pair_permutation(nzb)
        plan, strip_cols = _plan(nzb[perm])
        nc = _build_program(plan, strip_cols)
        _CACHE[key] = (perm, plan, strip_cols, nc)
    perm, plan, strip_cols, nc = _CACHE[key]
    nzb_p = nzb[perm]

    # Masked weights, gathered per row strip in storage order (J-major).
    # Wm's zeros for absent 32x32 blocks make half-present 64x32 panels
    # correct with no special-casing.
    Wm = np.where(mask_nz, W, np.float32(0)).astype(np.float32)
    W4 = Wm.reshape(GI, BS, GJ, BS)  # block (i, j) = W4[i, :, j, :]
    nzb2 = nzb_p[0::2] | nzb_p[1::2]
    strips = {}
    for r2 in range(2):
        if strip_cols[r2] == 0:
            continue
        II, JJ = [], []
        for J in range(N_J):
            for I in range(GP):
                for j in range(J * JCOLS, (J + 1) * JCOLS):
                    if nzb2[I, j] and I % 2 == r2:
                        II.append(I)
                        JJ.append(j)
        II = np.asarray(II)
        JJ = np.asarray(JJ)
        top = W4[perm[2 * II], :, JJ, :]       # [n, 32, 32]
        bot = W4[perm[2 * II + 1], :, JJ, :]   # [n, 32, 32]
        panel = np.concatenate([top, bot], axis=1)  # [n, 64, 32]
        strips[r2] = np.ascontiguousarray(
            panel.transpose(1, 0, 2).reshape(2 * BS, -1)
        ).astype(BF16)

    xf = np.ascontiguousarray(x).reshape(B * S, IN_F)
    in_maps = []
    for c in range(N_CORES):
        xt = np.ascontiguousarray(
            xf[c * M_CORE : (c + 1) * M_CORE].T
        ).astype(BF16)
        xt = xt.reshape(GI, BS, M_CORE)[perm].reshape(IN_F, M_CORE)
        xtc = (
            xt.reshape(N_T, 128, N_MSL, MSL)
            .transpose(2, 0, 1, 3)
            .reshape(N_MSL * N_T, 128, MSL)
        )
        m = {"xt": np.ascontiguousarray(xtc)}
        for r2, arr in strips.items():
            m[f"w{r2}"] = arr
        in_maps.append(m)

    # --- run -------------------------------------------------------------
    res = run_bass_kernel_spmd(nc, in_maps, list(range(N_CORES)), trace=True)

    # --- host-side output assembly --------------------------------------
    y = np.empty((B * S, OUT_F), dtype=np.float32)
    for c in range(N_CORES):
        y[c * M_CORE : (c + 1) * M_CORE] = res.results[c]["out"].T
    y = y.reshape(B, S, OUT_F)
    if np.any(bias):
        # bias is all-zero in this problem's setup; handled host-side for
        # generality.
        y = y + bias.astype(np.float32)
    kernel.last_exec_time_ns = res.exec_time_ns
    return y


# revision 13
# speedup vs baseline: 1.2665x; 1.0177x over previous
"""Block-sparse linear kernel for Trainium2 (8 NeuronCores, SPMD data-parallel).

Computes y = x @ (W * mask) + bias for
    x    [8, 1024, 4096] f32
    W    [4096, 4096]    f32
    mask [4096, 4096]    int32 (32x32-block structured, ~25% block density)
    bias [4096]          f32
    y    [8, 1024, 4096] f32

Strategy
--------
- Data parallel: core c computes rows [1024c, 1024(c+1)) of the flattened
  [8192, 4096] activation (i.e. batch element c).
- The trn2 PE array is physically 16 independent 32x32 sub-arrays; we run it
  in 64x32 tiling mode (8 concurrent sub-arrays).  The mask's 32x32 block
  granularity maps onto vertical block pairs: each present 64x32 "super
  cell" (block rows 2I,2I+1 x block col j, present if either 32x32 block is
  nonzero) becomes one K=64/M=32/N=512 matmul on sub-array
  (row_grp=I%2, col_grp=j%4); fully-zero super cells are skipped.
- v2: each supercell's weights are loaded into the PE array ONCE and used
  for both 512-token m-slices back to back.  The tile legalizer splits every
  matmul into LDWEIGHTS+MATMUL (marking the matmul non-self-loading); a
  post-schedule pass deletes the second, redundant LDWEIGHTS of each pair
  after verifying (against the final PE instruction order) that the weights
  for that sub-array position are already loaded.  This halves weight-load
  traffic on the PE weight path and lets weights stream from HBM once
  (previously twice: once per m-slice pass).
- Ramp: the first GEN_J supertiles run in rounds of 2, their matmuls merged
  chunk-major (all supertiles' blocks for x chunk t before chunk t+1, both
  m-slices interleaved), so early compute tracks x-chunk DMA arrival instead
  of head-of-line blocking on a late chunk.  x chunks are DMA'd m0/m1
  interleaved per chunk to match.
- Weights are gathered host-side into per-row-strip BSR-style panels (this
  mirrors the nn.Module, which stores BSR values at init), cast to bf16;
  x is transposed/cast host-side.  All matmul FLOPs run in bf16 with fp32
  PSUM accumulation (measured rel. error ~2e-3).
- The device program is compiled against the observed block pattern; it is
  exact for arbitrary masks (any block containing a nonzero mask element is
  gathered with its W*mask values; absent blocks contribute exact zeros).
"""

import numpy as np
import ml_dtypes

B, S, IN_F, OUT_F = 8, 1024, 4096, 4096
BS = 32                      # sparsity block size
GI, GJ = IN_F // BS, OUT_F // BS
GP = GI // 2                 # vertical super-rows (64 rows each)
N_CORES = 8
M_CORE = (B * S) // N_CORES  # rows of x per core (1024)
MSL = 512                    # m-slice width (one PSUM bank of fp32)
N_MSL = M_CORE // MSL        # 2
JCOLS = 4                    # output block-columns per supertile (4*32 = 128 partitions)
N_J = GJ // JCOLS            # 32 output supertiles
N_T = IN_F // 128            # 32 xT tiles
GEN_J = 8                    # supertiles run chunk-major in rounds of 2

BF16 = ml_dtypes.bfloat16


def _ensure_ntff_hook():
    """Best-effort: make trace=True work under axon when the image's antenv
    lacks axon_hooks.  Harmless if it fails — tracing is skipped, results
    are still correct."""
    import sys, types
    try:
        import antenv  # noqa
    except ImportError:
        return
    try:
        from antenv.axon_hooks import get_axon_ntff_profile_hook
        if get_axon_ntff_profile_hook() is not None:
            return
        mod = sys.modules["antenv.axon_hooks"]
    except ImportError:
        mod = types.ModuleType("antenv.axon_hooks")
        mod._hook = None
        def set_axon_ntff_profile_hook(h, _m=mod):
            _m._hook = h
        def get_axon_ntff_profile_hook(_m=mod):
            return _m._hook
        mod.set_axon_ntff_profile_hook = set_axon_ntff_profile_hook
        mod.get_axon_ntff_profile_hook = get_axon_ntff_profile_hook
        sys.modules["antenv.axon_hooks"] = mod
        import antenv as _a
        _a.axon_hooks = mod
    try:
        from trn_agent_boot.trn_boot import _ntff_profile_via_ctypes
        mod.set_axon_ntff_profile_hook(
            _ntff_profile_via_ctypes("/opt/axon/libaxon_pjrt.so")
        )
    except Exception:
        pass


def _pair_permutation(nzb):
    """Order block-rows so vertically-paired rows co-occur in many columns.

    Greedy max-weight matching on C[a,b] = #columns where blocks a and b are
    both present; each matched pair becomes one 64-row super-row, so high
    weight = fewer half-empty 64x32 panels = fewer matmuls.
    """
    C = nzb.astype(np.int32) @ nzb.astype(np.int32).T
    pairs = []
    try:
        import networkx as nx
        G = nx.Graph()
        for a in range(GI):
            for b in range(a + 1, GI):
                G.add_edge(a, b, weight=int(C[a, b]))
        pairs = [
            (int(min(a, b)), int(max(a, b)))
            for a, b in nx.max_weight_matching(G, maxcardinality=True)
        ]
    except Exception:
        pass
    if len(pairs) != GI // 2:
        pairs = []
        iu = np.triu_indices(GI, k=1)
        order = np.argsort(C[iu])[::-1]
        used = np.zeros(GI, dtype=bool)
        for idx in order:
            a, b = iu[0][idx], iu[1][idx]
            if not used[a] and not used[b]:
                used[a] = used[b] = True
                pairs.append((int(a), int(b)))
                if len(pairs) == GI // 2:
                    break
    perm = []
    for a, b in pairs:
        perm.extend((a, b))
    for a in range(GI):      # safety for odd leftovers
        if a not in perm:
            perm.append(a)
    return np.asarray(perm)


def _plan(nzb):
    """Per-supertile weight storage layout and MM schedule (64x32 pairing).

    nzb: bool [GI, GJ] — which 32x32 blocks are present (in permuted row
    order).

    Returns (plan, strip_cols):
      plan[J] = {
        'chunks': {r2: (src_col_base, n_cells)},            # DMA per row strip
        'sched':  [(r2, c, woff_or_None, I, start, stop)],
      }
      strip_cols[r2] = total columns of strip r2's DRAM panel (r2 in {0,1}).
    woff None => dummy matmul with the zero-weight tile (region had no cells
    but must be initialized so the bank reduce reads defined values).
    """
    nzb2 = nzb[0::2] | nzb[1::2]       # [GP, GJ] supercell presence
    plan = []
    strip_cols = [0, 0]
    for J in range(N_J):
        per_strip = {0: [], 1: []}     # storage order: x-tile-ascending so the
        for I in range(GP):            # ramp consumes x chunks as they arrive
            for j in range(J * JCOLS, (J + 1) * JCOLS):
                if nzb2[I, j]:
                    per_strip[I % 2].append((I, j))
        chunks = {}
        queues = {}                    # (r2, c) -> list of (r2, c, woff, I)
        for r2 in range(2):
            cells = per_strip[r2]
            chunks[r2] = (strip_cols[r2], len(cells))
            strip_cols[r2] += len(cells) * BS
            for k, (I, j) in enumerate(cells):
                c = j % 4
                queues.setdefault((r2, c), []).append((r2, c, k * BS, I))
        for r2 in range(2):
            for c in range(4):
                if (r2, c) not in queues:
                    queues[(r2, c)] = [(r2, c, None, 0)]
        # Round-robin across the 8 sub-array positions for concurrency,
        # alternating row groups so consecutive weight loads target
        # different halves of the PE array (deeper load pull-ahead).
        sched = []
        qorder = [(0, 0), (1, 0), (0, 1), (1, 1), (0, 2), (1, 2), (0, 3), (1, 3)]
        qlists = [queues[k] for k in qorder]
        idx = [0] * len(qlists)
        remaining = sum(len(q) for q in qlists)
        while remaining:
            for qi, q in enumerate(qlists):
                if idx[qi] < len(q):
                    r2, c, woff, I = q[idx[qi]]
                    start = idx[qi] == 0
                    stop = idx[qi] == len(q) - 1
                    sched.append((r2, c, woff, I, start, stop))
                    idx[qi] += 1
                    remaining -= 1
        plan.append({"chunks": chunks, "sched": sched})
    return plan, strip_cols


def _elide_redundant_ldweights(nc, candidates):
    """Delete LDWEIGHTS whose weights are provably already loaded.

    Walks each basic block's final (scheduled) PE instruction stream,
    tracking per tile_position the access-pattern of the last kept
    LDWEIGHTS.  An LDWEIGHTS is deleted iff (a) the matmul it precedes is a
    marked candidate (the m1 twin of an identically-weighted m0 matmul) and
    (b) the tracked state for its position already equals its weights AP.
    Waits/updates on a deleted LDWEIGHTS move onto its matmul; dep-graph
    descendant references are repointed.  This is order-verified: if the
    scheduler separated a pair, the state check fails and the load is kept.
    """
    import concourse.mybir as mybir

    n_removed = 0
    n_kept_cand = 0
    renames = {}
    for bb in nc.main_func.blocks:
        insts = list(bb.instructions)
        pe = [
            (i, x)
            for i, x in enumerate(insts)
            if x.engine == mybir.EngineType.PE
        ]
        state = {}
        dead = []
        for k, (idx, inst) in enumerate(pe):
            if not isinstance(inst, mybir.InstLdweights):
                continue
            pos = inst.tile_position
            aps = str(inst.ins[0])
            mm = pe[k + 1][1] if k + 1 < len(pe) else None
            if (
                mm is not None
                and type(mm).__name__ == "InstMatmult"
                and mm.name in candidates
            ):
                if state.get(pos) == aps:
                    si = inst.sync_info
                    if si is not None and (si.on_wait or si.on_update):
                        msi = mm.sync_info
                        if msi is None:
                            mm.sync_info = mybir.SyncInfo(
                                on_wait=list(si.on_wait),
                                on_update=list(si.on_update),
                            )
                        else:
                            mm.sync_info = mybir.SyncInfo(
                                on_wait=list(si.on_wait) + list(msi.on_wait),
                                on_update=list(msi.on_update)
                                + list(si.on_update),
                            )
                    dead.append((idx, inst))
                    renames[inst.name] = mm.name
                    continue
                n_kept_cand += 1
            state[pos] = aps
        for idx, inst in sorted(dead, key=lambda t: -t[0]):
            del bb.instructions[idx]
            nc.inst_map.pop(inst.name, None)
            n_removed += 1
    if renames:
        dead_names = set(renames)
        for name, inst in nc.inst_map.items():
            d = inst.descendants
            if d:
                hit = dead_names.intersection(d)
                for old in hit:
                    d.discard(old)
                    d.add(renames[old])
    return n_removed, n_kept_cand


def _build_program(plan, strip_cols):
    import concourse.bacc as bacc
    import concourse.tile as tile
    import concourse.mybir as mybir

    nc = bacc.Bacc(debug=False)
    bf16, f32 = mybir.dt.bfloat16, mybir.dt.float32

    xt_d = nc.declare_dram_parameter(
        "xt", [N_MSL * N_T, 128, MSL], bf16, isOutput=False
    )
    w_d = {}
    for r2 in range(2):
        if strip_cols[r2] > 0:
            w_d[r2] = nc.declare_dram_parameter(
                f"w{r2}", [2 * BS, strip_cols[r2]], bf16, isOutput=False
            )
    out_d = nc.declare_dram_parameter("out", [OUT_F, M_CORE], f32, isOutput=True)

    # Largest per-(J, strip) weight chunk, in columns (>= BS for the tile alloc).
    lmax = BS
    for p in plan:
        for r2 in range(2):
            lmax = max(lmax, p["chunks"][r2][1] * BS)

    elide = set()

    with tile.TileContext(nc) as tc:
        with (
            tc.tile_pool(name="xp", bufs=1) as xp,
            tc.tile_pool(name="zp", bufs=1) as zp,
            tc.tile_pool(name="wp", bufs=10) as wp,
            tc.tile_pool(name="ep", bufs=10) as ep,
            tc.tile_pool(name="pp", bufs=4, space="PSUM") as pp,
        ):
            def load_w(J, engs=None):
                wt = wp.tile([128, lmax], bf16, tag="wt")
                for r2 in range(2):
                    base, ncell = plan[J]["chunks"][r2]
                    if ncell:
                        eng = nc.scalar if engs is None else engs[r2 % len(engs)]
                        eng.dma_start(
                            wt[64 * r2 : 64 * r2 + 64, : ncell * BS],
                            w_d[r2][:, base : base + ncell * BS],
                        )
                return wt

            Xc = {}

            def load_x_chunk(t, m, eng):
                xchunk = xp.tile([128, MSL], bf16, tag=f"x{t}_{m}")
                Xc[(t, m)] = xchunk
                eng.dma_start(xchunk[:], xt_d[m * N_T + t])

            # DMA emission order: the GEN supertiles' weights lead, then ALL
            # of x m-slice 0 round-robin across the three input queues (the
            # m0 generation sweep tracks its arrival), then x m-slice 1, then
            # the remaining weights.  Per-queue order follows emission.
            QS = (nc.sync, nc.gpsimd, nc.scalar)
            zw = zp.tile([128, BS], bf16)
            nc.vector.memset(zw[:], 0.0)
            wts = {}
            for J in range(4):
                wts[J] = load_w(J, engs=(QS[J % 3], QS[(J + 1) % 3]))
            for m in range(N_MSL):
                for t in range(N_T):
                    load_x_chunk(t, m, QS[t % 3])
            for J in range(4, N_J):
                wts[J] = load_w(J, engs=(QS[J % 3], QS[(J + 1) % 3]))

            def emit_mm(P, wt, r2, c, woff, I, m, start, stop):
                lhsT = (
                    zw[64 * r2 : 64 * r2 + 64, :]
                    if woff is None
                    else wt[64 * r2 : 64 * r2 + 64, woff : woff + BS]
                )
                return nc.tensor.matmul(
                    P[32 * c : 32 * c + 32, r2, :],
                    lhsT,
                    Xc[(I // 2, m)][64 * r2 : 64 * r2 + 64, :],
                    start=start,
                    stop=stop,
                    tile_position=(64 * r2, 32 * c),
                )

            n_evac = [0]

            def emit_evac(P, J, m):
                ob = ep.tile([128, MSL], f32, tag="ob")
                nc.vector.reduce_sum(
                    ob[:], P[:].transpose([0, 2, 1]), axis=mybir.AxisListType.X
                )
                (nc.sync, nc.gpsimd)[n_evac[0] % 2].dma_start(
                    out_d[128 * J : 128 * (J + 1), m * MSL : (m + 1) * MSL],
                    ob[:],
                )
                n_evac[0] += 1

            # GEN: the first 4 supertiles' m-slice sweeps run chunk-major
            # merged (all four tiles' blocks for x chunk t before any of
            # chunk t+1), so early compute tracks x-chunk DMA arrival.  The
            # m0 sweep runs first (x m0 is DMA'd first), then the m1 sweep
            # (self-loading: array weights were clobbered in between).
            GEN = list(range(4))
            merged = []
            for J in GEN:
                for k, (r2, c, woff, I, _s0, _s1) in enumerate(plan[J]["sched"]):
                    t = -1 if woff is None else I // 2
                    merged.append((t, k, J, r2, c, woff, I))
            merged.sort(key=lambda e: (e[0], e[1], e[2]))
            first_of = {}
            last_of = {}
            for idx, e in enumerate(merged):
                key = (e[2], e[3], e[4])
                first_of.setdefault(key, idx)
                last_of[key] = idx
            for m in range(N_MSL):
                Pg = {
                    J: pp.tile([128, 2, MSL], f32, tag="P", name=f"Pg{m}_{J}")
                    for J in GEN
                }
                for idx, (t, k, J, r2, c, woff, I) in enumerate(merged):
                    key = (J, r2, c)
                    emit_mm(
                        Pg[J], wts[J], r2, c, woff, I, m,
                        first_of[key] == idx, last_of[key] == idx,
                    )
                for J in GEN:
                    emit_evac(Pg[J], J, m)

            # Steady phase: per supertile, each supercell's weights are
            # loaded into the PE array once; the m1 matmul is emitted LAG
            # entries after its m0 twin so it lands on a different sub-array
            # position (positions rotate with period 8) — back-to-back
            # same-position matmuls would serialize on the sub-array, since
            # matmul starts are pc-monotone.  LAG < 8 keeps the weight state
            # intact for the elision pass.
            LAG = 6
            pend = []

            def pop_m1():
                P1, wt, (r2, c, woff, I, st, sp) = pend.pop(0)
                mm1 = emit_mm(P1, wt, r2, c, woff, I, 1, st, sp)
                elide.add(mm1.ins.name)

            for J in range(len(GEN), N_J):
                P0 = pp.tile([128, 2, MSL], f32, tag="P", name=f"P0_{J}")
                P1 = pp.tile([128, 2, MSL], f32, tag="P", name=f"P1_{J}")
                for entry in plan[J]["sched"]:
                    r2, c, woff, I, start, stop = entry
                    # A pending m1 at this entry's position would lose its
                    # array weights to this entry's load — emit it first.
                    while pend and any(
                        (p[2][0], p[2][1]) == (r2, c) for p in pend
                    ):
                        pop_m1()
                    emit_mm(P0, wts[J], r2, c, woff, I, 0, start, stop)
                    pend.append((P1, wts[J], entry))
                    if len(pend) > LAG:
                        pop_m1()
                while pend:
                    pop_m1()
                emit_evac(P0, J, 0)
                emit_evac(P1, J, 1)

    n_removed, n_kept = _elide_redundant_ldweights(nc, elide)
    _build_program.elide_stats = (n_removed, n_kept, len(elide))
    print(
        f"[kernel] ldweights elided {n_removed}, kept-candidates {n_kept}, "
        f"candidates {len(elide)}"
    )
    nc.compile()
    return nc


_CACHE = {}


def kernel(x, W, bias, mask):
    assert x.shape == (B, S, IN_F) and W.shape == (IN_F, OUT_F)
    _ensure_ntff_hook()
    from concourse.bass_utils import run_bass_kernel_spmd

    # --- host-side input prep -------------------------------------------
    mask_nz = mask != 0
    nzb = np.asarray(mask_nz.reshape(GI, BS, GJ, BS).any(axis=(1, 3)))

    key = nzb.tobytes()
    if key not in _CACHE:
        perm = _pair_permutation(nzb)
        plan, strip_cols = _plan(nzb[perm])
        nc = _build_program(plan, strip_cols)
        _CACHE[key] = (perm, plan, strip_cols, nc)
    perm, plan, strip_cols, nc = _CACHE[key]
    nzb_p = nzb[perm]

    # Masked weights, gathered per row strip in storage order (J-major).
    # Wm's zeros for absent 32x32 blocks make half-present 64x32 panels
    # correct with no special-casing.
    Wm = np.where(mask_nz, W, np.float32(0)).astype(np.float32)
    W4 = Wm.reshape(GI, BS, GJ, BS)  # block (i, j) = W4[i, :, j, :]
    nzb2 = nzb_p[0::2] | nzb_p[1::2]
    strips = {}
    for r2 in range(2):
        if strip_cols[r2] == 0:
            continue
        II, JJ = [], []
        for J in range(N_J):
            for I in range(GP):
                for j in range(J * JCOLS, (J + 1) * JCOLS):
                    if nzb2[I, j] and I % 2 == r2:
                        II.append(I)
                        JJ.append(j)
        II = np.asarray(II)
        JJ = np.asarray(JJ)
        top = W4[perm[2 * II], :, JJ, :]       # [n, 32, 32]
        bot = W4[perm[2 * II + 1], :, JJ, :]   # [n, 32, 32]
        panel = np.concatenate([top, bot], axis=1)  # [n, 64, 32]
        strips[r2] = np.ascontiguousarray(
            panel.transpose(1, 0, 2).reshape(2 * BS, -1)
        ).astype(BF16)

    xf = np.ascontiguousarray(x).reshape(B * S, IN_F)
    in_maps = []
    for c in range(N_CORES):
        xt = np.ascontiguousarray(
            xf[c * M_CORE : (c + 1) * M_CORE].T
        ).astype(BF16)
        xt = xt.reshape(GI, BS, M_CORE)[perm].reshape(IN_F, M_CORE)
        xtc = (
            xt.reshape(N_T, 128, N_MSL, MSL)
            .transpose(2, 0, 1, 3)
            .reshape(N_MSL * N_T, 128, MSL)
        )
        m = {"xt": np.ascontiguousarray(xtc)}
        for r2, arr in strips.items():
            m[f"w{r2}"] = arr
        in_maps.append(m)

    # --- run -------------------------------------------------------------
    res = run_bass_kernel_spmd(nc, in_maps, list(range(N_CORES)), trace=True)

    # --- host-side output assembly --------------------------------------
    y = np.empty((B * S, OUT_F), dtype=np.float32)
    for c in range(N_CORES):
        y[c * M_CORE : (c + 1) * M_CORE] = res.results[c]["out"].T
    y = y.reshape(B, S, OUT_F)
    if np.any(bias):
        # bias is all-zero in this problem's setup; handled host-side for
        # generality.
        y = y + bias.astype(np.float32)
    kernel.last_exec_time_ns = res.exec_time_ns
    return y
